# revision 41
# baseline (speedup 1.0000x reference)
"""Multi-head attention (B=4, N=2048, C=1024, H=16, D=64) on 8 Trainium2 cores.

Strategy: tensor-parallel over heads (2 heads per core). Each core:
  phase A: projects full x into qT/kT (layout [d, tokens], head hh on
           partitions hh*64..hh*64+63) and V' (layout [tokens, d+1] with a
           trailing ones column) for its 2 heads,
  phase B: transposed attention scores ST[k, q] = K Q^T, exp (no max
           subtraction -- scores are ~N(0,1), fp32-safe), then AV in the
           transposed orientation out^T[d+1, q] = V'^T @ P^T.  Row 64 of
           out^T is the softmax denominator (ones column).  Normalization:
           reciprocal of that row, PE-matmul broadcast across partitions,
           elementwise multiply.  Finally the per-core slice of the output
           projection; the 8 partial projections are summed on host (the
           "unshard" of a row-parallel linear).

Matmul operands are bf16 (1 PE cycle/row vs 4 for fp32) with fp32 PSUM
accumulation; softmax denominators/normalization stay fp32.
"""

import numpy as np
import ml_dtypes
from contextlib import ExitStack

import concourse.bass as bass
import concourse.mybir as mybir
import concourse.tile as tile
from concourse import bacc
from concourse import bass_utils

B, N, C = 4, 2048, 1024
H, D = 16, 64
T = B * N                 # 8192 tokens
NCORES = 8
HPC = H // NCORES         # heads per core = 2
SCALE = D ** -0.5

F32 = mybir.dt.float32
BF16 = mybir.dt.bfloat16

TS = 512                  # phase-A token tile (free dim)
NTS = T // TS             # 16
CCN = C // 128            # 8 contraction chunks
KC = N // 128             # 16 key chunks per batch
QB = N // 512             # 4 query blocks per batch


def _build_graph(nb=B):
    nc = bacc.Bacc("TRN2", target_bir_lowering=False, debug=False,
                   num_devices=NCORES)
    xT = nc.dram_tensor("xT", [C, T], BF16, kind="ExternalInput").ap()
    # wqk columns: [q_h0 | q_h1 | k_h0 | k_h1], each D wide
    wqk = nc.dram_tensor("wqk", [C, HPC * 2 * D], BF16, kind="ExternalInput").ap()
    wv = nc.dram_tensor("wv", [C, HPC * D], BF16, kind="ExternalInput").ap()
    wp = nc.dram_tensor("wp", [HPC * D, C], BF16, kind="ExternalInput").ap()
    y = nc.dram_tensor("y", [T, C], BF16, kind="ExternalOutput").ap()

    with tile.TileContext(nc) as tc, ExitStack() as ctx:
        const = ctx.enter_context(tc.tile_pool(name="const", bufs=1))
        xpool = ctx.enter_context(tc.tile_pool(name="x", bufs=5))
        probs = ctx.enter_context(tc.tile_pool(name="probs", bufs=3))
        stage = ctx.enter_context(tc.tile_pool(name="stage", bufs=3))
        attnp = ctx.enter_context(tc.tile_pool(name="attn", bufs=6))
        alop = ctx.enter_context(tc.tile_pool(name="attn_lo", bufs=2))
        rpool = ctx.enter_context(tc.tile_pool(name="recip", bufs=2))
        outp = ctx.enter_context(tc.tile_pool(name="out", bufs=4))
        # PSUM: st (2 banks x 2) + av (1 x 2) + shared mm (1 x 2) = 8 banks
        st_psum = ctx.enter_context(
            tc.tile_pool(name="st", bufs=2, space="PSUM"))
        av_psum = ctx.enter_context(
            tc.tile_pool(name="av", bufs=2, space="PSUM"))
        mm_psum = ctx.enter_context(
            tc.tile_pool(name="mm", bufs=2, space="PSUM"))

        ones = const.tile([65, 64], BF16)
        nc.gpsimd.memset(ones[:], 1.0)

        wqk_sb = const.tile([128, CCN, HPC * 2 * D], BF16)
        wv_sb = const.tile([128, CCN, HPC * D], BF16)
        wp_sb = const.tile([128, C], BF16)
        for cc in range(CCN):
            nc.gpsimd.dma_start(wqk_sb[:, cc, :], wqk[cc * 128:(cc + 1) * 128, :])
            nc.gpsimd.dma_start(wv_sb[:, cc, :], wv[cc * 128:(cc + 1) * 128, :])
        nc.gpsimd.dma_start(wp_sb[:], wp[:, :])

        # per-batch phase-A outputs (separate tiles so phase B of batch b
        # can overlap phase A of batch b+1):
        #   qT/kT: [d, tokens]; head hh lives on partitions hh*64..+63
        #   v: per head, 16 key-tiles of [128 tok, 65] (col 64 = 1.0)
        qT_b, kT_b, v_b = [], [], []
        for b in range(B):
            qT_b.append(const.tile([128, N], BF16, name=f"qTb{b}", tag=f"qT{b}"))
            kT_b.append(const.tile([128, N], BF16, name=f"kTb{b}", tag=f"kT{b}"))
            v_b.append(const.tile([128, HPC, KC, D + 1], BF16, name=f"vb{b}", tag=f"v{b}"))
            nc.gpsimd.memset(v_b[b][:, :, :, D:D + 1], 1.0)

        def phase_A(b, ts):
            # ---- phase A(b) chunk ts: projections for batch b ----
            if True:
                sl = slice(ts * TS, (ts + 1) * TS)
                gsl = slice(b * N + ts * TS, b * N + (ts + 1) * TS)
                xt = xpool.tile([128, CCN, TS], BF16, tag="x")
                # two 512KB DMAs per token tile: (p, cc, j) <- xT[cc*128+p, g0+j]
                xTr = xT.rearrange("(cc p) t -> p cc t", p=128)
                nc.sync.dma_start(xt[:, 0:4, :], xTr[:, 0:4, gsl])
                if b == 0:
                    nc.scalar.dma_start(xt[:, 4:8, :], xTr[:, 4:8, gsl])
                else:
                    nc.sync.dma_start(xt[:, 4:8, :], xTr[:, 4:8, gsl])
                xts = [xt[:, cc, :] for cc in range(CCN)]
                for qk_i, dst in ((0, qT_b[b]), (1, kT_b[b])):
                    ps = mm_psum.tile([128, TS], F32, tag="mm")
                    for cc in range(CCN):
                        nc.tensor.matmul(
                            ps[:],
                            wqk_sb[:, cc, qk_i * 128:(qk_i + 1) * 128],
                            xts[cc],
                            start=(cc == 0), stop=(cc == CCN - 1))
                    nc.vector.tensor_copy(dst[:, sl], ps[:])
                for j in range(TS // 128):
                    vp = mm_psum.tile([128, HPC * D], F32, tag="mm")
                    for cc in range(CCN):
                        nc.tensor.matmul(
                            vp[:],
                            xt[:, cc, j * 128:(j + 1) * 128],
                            wv_sb[:, cc, :],
                            start=(cc == 0), stop=(cc == CCN - 1))
                    for hh in range(HPC):
                        nc.vector.tensor_copy(
                            v_b[b][:, hh, ts * 4 + j, 0:D],
                            vp[:, hh * D:(hh + 1) * D])

        def phase_B(b, qb):
            # ---- phase B(b) query block qb (attention only; proj deferred)
            if True:
                q0 = qb * 512
                attn = attnp.tile([128, 512], BF16, tag="attn")
                attn_tiles[(b, qb)] = attn
                for hh in range(HPC):
                    h0 = hh * 64
                    av_t = av_psum.tile([D + 1, 512], F32, tag="av")
                    for kp in range(KC // 2):
                        st = st_psum.tile([128, 2, 512], F32, tag="st")
                        pb = probs.tile([128, 2, 512], BF16, tag="probs")
                        for half in range(2):
                            k0 = (kp * 2 + half) * 128
                            nc.tensor.matmul(
                                st[:, half, :],
                                kT_b[b][h0:h0 + 64, k0:k0 + 128],
                                qT_b[b][h0:h0 + 64, q0:q0 + 512],
                                start=True, stop=True)
                        nc.scalar.activation(
                            pb[:], st[:], mybir.ActivationFunctionType.Exp,
                            scale=SCALE)
                        for half in range(2):
                            nc.tensor.matmul(
                                av_t[:],
                                v_b[b][:, hh, kp * 2 + half, :],
                                pb[:, half, :],
                                start=(kp == 0 and half == 0),
                                stop=(kp == KC // 2 - 1 and half == 1))
                    rcb = rpool.tile([D + 1, 512], BF16, tag="recipb")
                    with nc.allow_low_precision("softmax denom in bf16"):
                        nc.vector.reciprocal(rcb[D:D + 1, :], av_t[D:D + 1, :])
                    bc = av_psum.tile([64, 512], F32, tag="av")
                    nc.tensor.matmul(bc[:], ones[D:D + 1, :],
                                     rcb[D:D + 1, :],
                                     start=True, stop=True)
                    sg = stage.tile([64, 512], F32, tag="stage")
                    nc.vector.tensor_copy(sg[:], av_t[0:D, :])
                    if hh == 0:
                        nc.vector.tensor_mul(attn[0:64, :], sg[:], bc[:])
                    else:
                        alo = alop.tile([64, 512], BF16, tag="attn_lo")
                        nc.vector.tensor_mul(alo[:], sg[:], bc[:])
                        # engines are partition-aligned; DMA moves the head-1
                        # rows up to partitions 64-127
                        nc.sync.dma_start(attn[h0:h0 + 64, :], alo[:])
        def phase_P(b):
            # ---- deferred output projection for batch b: a dense PE
            # stretch that overlaps the next batch's ACT-bound start ----
            for qb in range(QB):
                attn = attn_tiles.pop((b, qb))
                q0 = qb * 512
                for tt in range(4):
                    ot = outp.tile([128, C], BF16, tag="out")
                    for ob in range(2):
                        pp = mm_psum.tile([128, 512], F32, tag="mm")
                        nc.tensor.matmul(
                            pp[:],
                            attn[:, tt * 128:(tt + 1) * 128],
                            wp_sb[:, ob * 512:(ob + 1) * 512],
                            start=True, stop=True)
                        nc.vector.tensor_copy(ot[:, ob * 512:(ob + 1) * 512],
                                              pp[:])
                    nc.sync.dma_start(
                        y[b * N + q0 + tt * 128:b * N + q0 + (tt + 1) * 128, :],
                        ot[:])

        # software-pipelined emission: keep phase A one batch ahead,
        # interleaved per query-block so its PE work fills the ACT-bound
        # bubbles of phase B uniformly
        attn_tiles = {}
        for ts in range(N // TS):
            phase_A(0, ts)
        for b in range(nb):
            for qb in range(QB):
                if b + 1 < nb:
                    phase_A(b + 1, qb)
                phase_B(b, qb)
            phase_P(b)

    nc.compile()
    return nc


_NC = None
LAST_EXEC_NS = None


def _get_nc():
    global _NC
    if _NC is None:
        _NC = _build_graph()
    return _NC


def _make_in_maps(x, W_qkv, W_proj):
    bf = ml_dtypes.bfloat16
    xT = np.ascontiguousarray(x.reshape(T, C).T.astype(bf))
    in_maps = []
    for i in range(NCORES):
        h0 = HPC * i
        # columns: q_h0 | q_h1 | k_h0 | k_h1
        wqk_i = np.concatenate(
            [W_qkv[(h0 + hh) * D:(h0 + hh + 1) * D, :].T for hh in range(HPC)]
            + [W_qkv[C + (h0 + hh) * D:C + (h0 + hh + 1) * D, :].T
               for hh in range(HPC)],
            axis=1)                                   # [C, HPC*2*D]
        wv_i = W_qkv[2 * C + h0 * D:2 * C + (h0 + HPC) * D, :].T  # [C, HPC*D]
        wp_i = W_proj[:, h0 * D:(h0 + HPC) * D].T     # [HPC*D, C]
        in_maps.append({
            "xT": xT,
            "wqk": np.ascontiguousarray(wqk_i.astype(bf)),
            "wv": np.ascontiguousarray(wv_i.astype(bf)),
            "wp": np.ascontiguousarray(wp_i.astype(bf)),
        })
    return in_maps


def kernel(x, W_qkv, W_proj, b_proj, trace=False):
    global LAST_EXEC_NS
    x = np.ascontiguousarray(np.asarray(x, dtype=np.float32))
    W_qkv = np.asarray(W_qkv, dtype=np.float32)
    W_proj = np.asarray(W_proj, dtype=np.float32)
    b_proj = np.asarray(b_proj, dtype=np.float32)

    in_maps = _make_in_maps(x, W_qkv, W_proj)
    nc = _get_nc()
    res = None
    for attempt in range(3):
        try:
            res = bass_utils.run_bass_kernel_spmd(
                nc, in_maps, core_ids=list(range(NCORES)), trace=trace)
            break
        except Exception:
            # transient "mesh desynced / NRT_EXEC_UNIT_UNRECOVERABLE" errors
            # clear on retry
            if attempt == 2:
                raise
            import time
            time.sleep(5)
    LAST_EXEC_NS = res.exec_time_ns
    acc = res.results[0]["y"].astype(np.float64)
    for i in range(1, NCORES):
        acc += res.results[i]["y"]
    out = (acc + b_proj).astype(np.float32)
    return out.reshape(B, N, C)


def bench(x, W_qkv, W_proj, b_proj, iters=10):
    """Device-resident repeat timing of the NEFF execution.

    Returns (per_iter_ns_blocking, per_iter_ns_pipelined, full output).
    """
    x = np.ascontiguousarray(np.asarray(x, dtype=np.float32))
    in_maps = _make_in_maps(x, np.asarray(W_qkv, dtype=np.float32),
                            np.asarray(W_proj, dtype=np.float32))
    t_block, t_pipe, y_percore = _bench_impl(in_maps, iters=iters)
    acc = y_percore[0].astype(np.float64)
    for i in range(1, NCORES):
        acc += y_percore[i]
    out = (acc + np.asarray(b_proj, dtype=np.float32)).astype(np.float32)
    return t_block, t_pipe, out.reshape(B, N, C)


def _bench_impl(in_maps, iters=10, nc=None):
    import time
    import jax
    from jax.experimental.shard_map import shard_map
    from jax.sharding import Mesh, PartitionSpec, NamedSharding
    from concourse import bass2jax, mybir as mb

    nc = nc or _get_nc()
    bass2jax.install_neuronx_cc_hook()

    partition_name = (nc.partition_id_tensor.name
                      if nc.partition_id_tensor else None)
    in_names, out_names, out_avals, zero_outs = [], [], [], []
    for alloc in nc.m.functions[0].allocations:
        if not isinstance(alloc, mb.MemoryLocationSet):
            continue
        name = alloc.memorylocations[0].name
        if alloc.kind == "ExternalInput":
            if name != partition_name:
                in_names.append(name)
        elif alloc.kind == "ExternalOutput":
            out_names.append(name)
            shape = tuple(alloc.tensor_shape)
            dtype = mb.dt.np(alloc.dtype)
            out_avals.append(jax.core.ShapedArray(shape, dtype))
            zero_outs.append(np.zeros(shape, dtype))
    n_params = len(in_names)
    all_names = in_names + out_names
    if partition_name is not None:
        all_names = all_names + [partition_name]

    def _body(*args):
        operands = list(args)
        if partition_name is not None:
            operands.append(bass2jax.partition_id_tensor())
        outs = bass2jax._bass_exec_p.bind(
            *operands,
            out_avals=tuple(out_avals),
            in_names=tuple(all_names),
            out_names=tuple(out_names),
            lowering_input_output_aliases=(),
            sim_require_finite=True,
            sim_require_nnan=True,
            nc=nc,
        )
        return tuple(outs)

    devices = jax.devices()[:NCORES]
    mesh = Mesh(np.asarray(devices), ("core",))
    spec = PartitionSpec("core")
    sharded = jax.jit(
        shard_map(_body, mesh=mesh,
                  in_specs=(spec,) * (n_params + len(out_names)),
                  out_specs=(spec,) * len(out_names),
                  check_rep=False),
        keep_unused=True)

    shd = NamedSharding(mesh, spec)
    concat_in = [
        np.concatenate([np.asarray(in_maps[c][nm]) for c in range(NCORES)],
                       axis=0) for nm in in_names]
    concat_zero = [np.zeros((NCORES * z.shape[0], *z.shape[1:]), z.dtype)
                   for z in zero_outs]
    dev_in = [jax.device_put(a, shd) for a in concat_in]
    dev_zero = [jax.device_put(a, shd) for a in concat_zero]

    out = sharded(*dev_in, *dev_zero)           # warm-up / compile
    jax.block_until_ready(out)
    if iters == 0:
        return (sharded, dev_in, dev_zero, out_names)

    t_block = []
    for _ in range(iters):
        t0 = time.perf_counter()
        out = sharded(*dev_in, *dev_zero)
        jax.block_until_ready(out)
        t_block.append(time.perf_counter() - t0)

    t0 = time.perf_counter()
    outs = [sharded(*dev_in, *dev_zero) for _ in range(iters)]
    jax.block_until_ready(outs)
    t_pipe = (time.perf_counter() - t0) / iters

    y_global = np.asarray(out[out_names.index("y")])
    return (min(t_block) * 1e9, t_pipe * 1e9,
            y_global.reshape(NCORES, -1, y_global.shape[-1]))



# revision 48
# speedup vs baseline: 1.2716x; 1.2716x over previous
"""Multi-head attention (B=4, N=2048, C=1024, H=16, D=64) on 8 Trainium2 cores.

Strategy: tensor-parallel over heads (2 heads per core). Each core:
  phase A: projects full x into qT/kT (layout [d, tokens], head hh on
           partitions hh*64..hh*64+63) and V' (layout [tokens, d+1] with a
           trailing ones column) for its 2 heads,
  phase B: transposed attention scores ST[k, q] = K Q^T, exp (no max
           subtraction -- scores are ~N(0,1), fp32-safe), then AV in the
           transposed orientation out^T[d+1, q] = V'^T @ P^T.  Row 64 of
           out^T is the softmax denominator (ones column).  Normalization:
           reciprocal of that row, PE-matmul broadcast across partitions,
           elementwise multiply.  Finally the per-core slice of the output
           projection; the 8 partial projections are summed on host (the
           "unshard" of a row-parallel linear).

Matmul operands are bf16 (1 PE cycle/row vs 4 for fp32) with fp32 PSUM
accumulation; softmax denominators/normalization stay fp32.
"""

import numpy as np
import ml_dtypes
from contextlib import ExitStack

import concourse.bass as bass
import concourse.mybir as mybir
import concourse.tile as tile
from concourse import bacc
from concourse import bass_utils

B, N, C = 4, 2048, 1024
H, D = 16, 64
T = B * N                 # 8192 tokens
NCORES = 8
HPC = H // NCORES         # heads per core = 2
SCALE = D ** -0.5

F32 = mybir.dt.float32
BF16 = mybir.dt.bfloat16

TS = 512                  # phase-A token tile (free dim)
NTS = T // TS             # 16
CCN = C // 128            # 8 contraction chunks
KC = N // 128             # 16 key chunks per batch
QB = N // 512             # 4 query blocks per batch


def _build_graph(nb=B):
    nc = bacc.Bacc("TRN2", target_bir_lowering=False, debug=False,
                   num_devices=NCORES)
    xT = nc.dram_tensor("xT", [C, T], BF16, kind="ExternalInput").ap()
    # wqk columns: [q_h0 | q_h1 | k_h0 | k_h1], each D wide
    wqk = nc.dram_tensor("wqk", [C, HPC * 2 * D], BF16, kind="ExternalInput").ap()
    wv = nc.dram_tensor("wv", [C, HPC * D], BF16, kind="ExternalInput").ap()
    wp = nc.dram_tensor("wp", [HPC * D, C], BF16, kind="ExternalInput").ap()
    y = nc.dram_tensor("y", [T, C], BF16, kind="ExternalOutput").ap()

    with tile.TileContext(nc) as tc, ExitStack() as ctx:
        const = ctx.enter_context(tc.tile_pool(name="const", bufs=1))
        xpool = ctx.enter_context(tc.tile_pool(name="x", bufs=4))
        probs = ctx.enter_context(tc.tile_pool(name="probs", bufs=3))
        stage = ctx.enter_context(tc.tile_pool(name="stage", bufs=3))
        attnp = ctx.enter_context(tc.tile_pool(name="attn", bufs=6))
        alop = ctx.enter_context(tc.tile_pool(name="attn_lo", bufs=2))
        rpool = ctx.enter_context(tc.tile_pool(name="recip", bufs=2))
        outp = ctx.enter_context(tc.tile_pool(name="out", bufs=4))
        # PSUM: st (2 banks x 2) + av (1 x 2) + shared mm (1 x 2) = 8 banks
        st_psum = ctx.enter_context(
            tc.tile_pool(name="st", bufs=2, space="PSUM"))
        av_psum = ctx.enter_context(
            tc.tile_pool(name="av", bufs=2, space="PSUM"))
        mm_psum = ctx.enter_context(
            tc.tile_pool(name="mm", bufs=2, space="PSUM"))

        ones = const.tile([65, 64], BF16)
        nc.gpsimd.memset(ones[:], 1.0)

        wqk_sb = const.tile([128, CCN, HPC * 2 * D], BF16)
        wv_sb = const.tile([128, CCN, HPC * D], BF16)
        wp_sb = const.tile([128, C], BF16)
        for cc in range(CCN):
            nc.gpsimd.dma_start(wqk_sb[:, cc, :], wqk[cc * 128:(cc + 1) * 128, :])
            nc.gpsimd.dma_start(wv_sb[:, cc, :], wv[cc * 128:(cc + 1) * 128, :])
        nc.gpsimd.dma_start(wp_sb[:], wp[:, :])

        # per-batch phase-A outputs (separate tiles so phase B of batch b
        # can overlap phase A of batch b+1):
        #   qT/kT: [d, tokens]; head hh lives on partitions hh*64..+63
        #   v: per head, 16 key-tiles of [128 tok, 65] (col 64 = 1.0)
        qT_b, kT_b, v_b = [], [], []
        for b in range(B):
            qT_b.append(const.tile([128, N], BF16, name=f"qTb{b}", tag=f"qT{b}"))
            kT_b.append(const.tile([128, N], BF16, name=f"kTb{b}", tag=f"kT{b}"))
            v_b.append(const.tile([128, HPC, KC, D + 1], BF16, name=f"vb{b}", tag=f"v{b}"))
            nc.gpsimd.memset(v_b[b][:, :, :, D:D + 1], 1.0)

        def phase_A_dma(b, ts):
            # ---- phase A(b) chunk ts: issue x loads (prefetch) ----
            gsl = slice(b * N + ts * TS, b * N + (ts + 1) * TS)
            xt_lo = xpool.tile([128, CCN // 2, TS], BF16,
                               name=f"xlo{b}_{ts}", tag="xlo")
            xt_hi = xpool.tile([128, CCN // 2, TS], BF16,
                               name=f"xhi{b}_{ts}", tag="xhi")
            # two 512KB DMAs per token tile: (p, cc, j) <- xT[cc*128+p, g0+j]
            xTr = xT.rearrange("(cc p) t -> p cc t", p=128)
            nc.sync.dma_start(xt_lo[:], xTr[:, 0:4, gsl])
            if b == 0:
                nc.scalar.dma_start(xt_hi[:], xTr[:, 4:8, gsl])
            else:
                nc.sync.dma_start(xt_hi[:], xTr[:, 4:8, gsl])
            x_tiles[(b, ts)] = (xt_lo, xt_hi)

        def phase_A(b, ts):
            # ---- phase A(b) chunk ts: projections for batch b ----
            if True:
                sl = slice(ts * TS, (ts + 1) * TS)
                xt_lo, xt_hi = x_tiles.pop((b, ts))
                xts = ([xt_lo[:, cc, :] for cc in range(CCN // 2)]
                       + [xt_hi[:, cc, :] for cc in range(CCN // 2)])
                for qk_i, dst in ((0, qT_b[b]), (1, kT_b[b])):
                    ps = mm_psum.tile([128, TS], F32, tag="mm")
                    for cc in range(CCN):
                        nc.tensor.matmul(
                            ps[:],
                            wqk_sb[:, cc, qk_i * 128:(qk_i + 1) * 128],
                            xts[cc],
                            start=(cc == 0), stop=(cc == CCN - 1))
                    nc.vector.tensor_copy(dst[:, sl], ps[:])
                for j in range(TS // 128):
                    vp = mm_psum.tile([128, HPC * D], F32, tag="mm")
                    for cc in range(CCN):
                        nc.tensor.matmul(
                            vp[:],
                            xts[cc][:, j * 128:(j + 1) * 128],
                            wv_sb[:, cc, :],
                            start=(cc == 0), stop=(cc == CCN - 1))
                    for hh in range(HPC):
                        nc.vector.tensor_copy(
                            v_b[b][:, hh, ts * 4 + j, 0:D],
                            vp[:, hh * D:(hh + 1) * D])

        def phase_B(b, qb):
            # ---- phase B(b) query block qb (attention only; proj deferred)
            if True:
                q0 = qb * 512
                attn = attnp.tile([128, 512], BF16, tag="attn")
                attn_tiles[(b, qb)] = attn
                for hh in range(HPC):
                    h0 = hh * 64
                    av_t = av_psum.tile([D + 1, 512], F32, tag="av")
                    for kp in range(KC // 2):
                        st = st_psum.tile([128, 2, 512], F32, tag="st")
                        pb = probs.tile([128, 2, 512], BF16, tag="probs")
                        for half in range(2):
                            k0 = (kp * 2 + half) * 128
                            nc.tensor.matmul(
                                st[:, half, :],
                                kT_b[b][h0:h0 + 64, k0:k0 + 128],
                                qT_b[b][h0:h0 + 64, q0:q0 + 512],
                                start=True, stop=True)
                        nc.scalar.activation(
                            pb[:], st[:], mybir.ActivationFunctionType.Exp,
                            scale=SCALE)
                        for half in range(2):
                            nc.tensor.matmul(
                                av_t[:],
                                v_b[b][:, hh, kp * 2 + half, :],
                                pb[:, half, :],
                                start=(kp == 0 and half == 0),
                                stop=(kp == KC // 2 - 1 and half == 1))
                    rcb = rpool.tile([D + 1, 512], BF16, tag="recipb")
                    with nc.allow_low_precision("softmax denom in bf16"):
                        nc.vector.reciprocal(rcb[D:D + 1, :], av_t[D:D + 1, :])
                    bc = av_psum.tile([64, 512], F32, tag="av")
                    nc.tensor.matmul(bc[:], ones[D:D + 1, :],
                                     rcb[D:D + 1, :],
                                     start=True, stop=True)
                    sg = stage.tile([64, 512], F32, tag="stage")
                    nc.vector.tensor_copy(sg[:], av_t[0:D, :])
                    if hh == 0:
                        nc.vector.tensor_mul(attn[0:64, :], sg[:], bc[:])
                    else:
                        alo = alop.tile([64, 512], BF16, tag="attn_lo")
                        nc.vector.tensor_mul(alo[:], sg[:], bc[:])
                        # engines are partition-aligned; DMA moves the head-1
                        # rows up to partitions 64-127
                        nc.sync.dma_start(attn[h0:h0 + 64, :], alo[:])
        def phase_P(b, only_qb=None):
            # ---- deferred output projection for batch b, interleaved into
            # the next batch so its PE work fills ACT-bound bubbles ----
            for qb in (range(QB) if only_qb is None else [only_qb]):
                attn = attn_tiles.pop((b, qb))
                q0 = qb * 512
                for tt in range(4):
                    ot = outp.tile([128, C], BF16, tag="out")
                    for ob in range(2):
                        pp = mm_psum.tile([128, 512], F32, tag="mm")
                        nc.tensor.matmul(
                            pp[:],
                            attn[:, tt * 128:(tt + 1) * 128],
                            wp_sb[:, ob * 512:(ob + 1) * 512],
                            start=True, stop=True)
                        nc.vector.tensor_copy(ot[:, ob * 512:(ob + 1) * 512],
                                              pp[:])
                    nc.sync.dma_start(
                        y[b * N + q0 + tt * 128:b * N + q0 + (tt + 1) * 128, :],
                        ot[:])

        # software-pipelined emission: keep phase A one batch ahead,
        # interleaved per query-block so its PE work fills the ACT-bound
        # bubbles of phase B uniformly
        attn_tiles = {}
        x_tiles = {}
        for ts in range(N // TS):
            phase_A_dma(0, ts)
            phase_A(0, ts)
        for b in range(nb):
            for qb in range(QB):
                if b + 1 < nb:
                    # issue the x loads one query-block ahead of their matmuls
                    phase_A_dma(b + 1, qb)
                    if qb > 0:
                        phase_A(b + 1, qb - 1)
                if b > 0:
                    phase_P(b - 1, only_qb=qb)
                phase_B(b, qb)
            if b + 1 < nb:
                phase_A(b + 1, QB - 1)
        phase_P(nb - 1)

    nc.compile()
    return nc


_NC = None
LAST_EXEC_NS = None


def _get_nc():
    global _NC
    if _NC is None:
        _NC = _build_graph()
    return _NC


def _make_in_maps(x, W_qkv, W_proj):
    bf = ml_dtypes.bfloat16
    xT = np.ascontiguousarray(x.reshape(T, C).T.astype(bf))
    in_maps = []
    for i in range(NCORES):
        h0 = HPC * i
        # columns: q_h0 | q_h1 | k_h0 | k_h1
        wqk_i = np.concatenate(
            [W_qkv[(h0 + hh) * D:(h0 + hh + 1) * D, :].T for hh in range(HPC)]
            + [W_qkv[C + (h0 + hh) * D:C + (h0 + hh + 1) * D, :].T
               for hh in range(HPC)],
            axis=1)                                   # [C, HPC*2*D]
        wv_i = W_qkv[2 * C + h0 * D:2 * C + (h0 + HPC) * D, :].T  # [C, HPC*D]
        wp_i = W_proj[:, h0 * D:(h0 + HPC) * D].T     # [HPC*D, C]
        in_maps.append({
            "xT": xT,
            "wqk": np.ascontiguousarray(wqk_i.astype(bf)),
            "wv": np.ascontiguousarray(wv_i.astype(bf)),
            "wp": np.ascontiguousarray(wp_i.astype(bf)),
        })
    return in_maps


def kernel(x, W_qkv, W_proj, b_proj, trace=False):
    global LAST_EXEC_NS
    x = np.ascontiguousarray(np.asarray(x, dtype=np.float32))
    W_qkv = np.asarray(W_qkv, dtype=np.float32)
    W_proj = np.asarray(W_proj, dtype=np.float32)
    b_proj = np.asarray(b_proj, dtype=np.float32)

    in_maps = _make_in_maps(x, W_qkv, W_proj)
    nc = _get_nc()
    res = None
    for attempt in range(3):
        try:
            res = bass_utils.run_bass_kernel_spmd(
                nc, in_maps, core_ids=list(range(NCORES)), trace=trace)
            break
        except Exception:
            # transient "mesh desynced / NRT_EXEC_UNIT_UNRECOVERABLE" errors
            # clear on retry
            if attempt == 2:
                raise
            import time
            time.sleep(5)
    LAST_EXEC_NS = res.exec_time_ns
    acc = res.results[0]["y"].astype(np.float64)
    for i in range(1, NCORES):
        acc += res.results[i]["y"]
    out = (acc + b_proj).astype(np.float32)
    return out.reshape(B, N, C)


def bench(x, W_qkv, W_proj, b_proj, iters=10):
    """Device-resident repeat timing of the NEFF execution.

    Returns (per_iter_ns_blocking, per_iter_ns_pipelined, full output).
    """
    x = np.ascontiguousarray(np.asarray(x, dtype=np.float32))
    in_maps = _make_in_maps(x, np.asarray(W_qkv, dtype=np.float32),
                            np.asarray(W_proj, dtype=np.float32))
    t_block, t_pipe, y_percore = _bench_impl(in_maps, iters=iters)
    acc = y_percore[0].astype(np.float64)
    for i in range(1, NCORES):
        acc += y_percore[i]
    out = (acc + np.asarray(b_proj, dtype=np.float32)).astype(np.float32)
    return t_block, t_pipe, out.reshape(B, N, C)


def _bench_impl(in_maps, iters=10, nc=None):
    import time
    import jax
    from jax.experimental.shard_map import shard_map
    from jax.sharding import Mesh, PartitionSpec, NamedSharding
    from concourse import bass2jax, mybir as mb

    nc = nc or _get_nc()
    bass2jax.install_neuronx_cc_hook()

    partition_name = (nc.partition_id_tensor.name
                      if nc.partition_id_tensor else None)
    in_names, out_names, out_avals, zero_outs = [], [], [], []
    for alloc in nc.m.functions[0].allocations:
        if not isinstance(alloc, mb.MemoryLocationSet):
            continue
        name = alloc.memorylocations[0].name
        if alloc.kind == "ExternalInput":
            if name != partition_name:
                in_names.append(name)
        elif alloc.kind == "ExternalOutput":
            out_names.append(name)
            shape = tuple(alloc.tensor_shape)
            dtype = mb.dt.np(alloc.dtype)
            out_avals.append(jax.core.ShapedArray(shape, dtype))
            zero_outs.append(np.zeros(shape, dtype))
    n_params = len(in_names)
    all_names = in_names + out_names
    if partition_name is not None:
        all_names = all_names + [partition_name]

    def _body(*args):
        operands = list(args)
        if partition_name is not None:
            operands.append(bass2jax.partition_id_tensor())
        outs = bass2jax._bass_exec_p.bind(
            *operands,
            out_avals=tuple(out_avals),
            in_names=tuple(all_names),
            out_names=tuple(out_names),
            lowering_input_output_aliases=(),
            sim_require_finite=True,
            sim_require_nnan=True,
            nc=nc,
        )
        return tuple(outs)

    devices = jax.devices()[:NCORES]
    mesh = Mesh(np.asarray(devices), ("core",))
    spec = PartitionSpec("core")
    sharded = jax.jit(
        shard_map(_body, mesh=mesh,
                  in_specs=(spec,) * (n_params + len(out_names)),
                  out_specs=(spec,) * len(out_names),
                  check_rep=False),
        keep_unused=True)

    shd = NamedSharding(mesh, spec)
    concat_in = [
        np.concatenate([np.asarray(in_maps[c][nm]) for c in range(NCORES)],
                       axis=0) for nm in in_names]
    concat_zero = [np.zeros((NCORES * z.shape[0], *z.shape[1:]), z.dtype)
                   for z in zero_outs]
    dev_in = [jax.device_put(a, shd) for a in concat_in]
    dev_zero = [jax.device_put(a, shd) for a in concat_zero]

    out = sharded(*dev_in, *dev_zero)           # warm-up / compile
    jax.block_until_ready(out)
    if iters == 0:
        return (sharded, dev_in, dev_zero, out_names)

    t_block = []
    for _ in range(iters):
        t0 = time.perf_counter()
        out = sharded(*dev_in, *dev_zero)
        jax.block_until_ready(out)
        t_block.append(time.perf_counter() - t0)

    t0 = time.perf_counter()
    outs = [sharded(*dev_in, *dev_zero) for _ in range(iters)]
    jax.block_until_ready(outs)
    t_pipe = (time.perf_counter() - t0) / iters

    y_global = np.asarray(out[out_names.index("y")])
    return (min(t_block) * 1e9, t_pipe * 1e9,
            y_global.reshape(NCORES, -1, y_global.shape[-1]))



# revision 55
# speedup vs baseline: 1.3181x; 1.0365x over previous
"""Multi-head attention (B=4, N=2048, C=1024, H=16, D=64) on 8 Trainium2 cores.

Strategy: tensor-parallel over heads (2 heads per core). Each core:
  phase A: projects full x into qT/kT (layout [d, tokens], head hh on
           partitions hh*64..hh*64+63) and V' (layout [tokens, d+1] with a
           trailing ones column) for its 2 heads,
  phase B: transposed attention scores ST[k, q] = K Q^T, exp (no max
           subtraction -- scores are ~N(0,1), fp32-safe), then AV in the
           transposed orientation out^T[d+1, q] = V'^T @ P^T.  Row 64 of
           out^T is the softmax denominator (ones column).  Normalization:
           reciprocal of that row, PE-matmul broadcast across partitions,
           elementwise multiply.  Finally the per-core slice of the output
           projection; the 8 partial projections are summed on host (the
           "unshard" of a row-parallel linear).

Matmul operands are bf16 (1 PE cycle/row vs 4 for fp32) with fp32 PSUM
accumulation; softmax denominators/normalization stay fp32.
"""

import numpy as np
import ml_dtypes
from contextlib import ExitStack

import concourse.bass as bass
import concourse.mybir as mybir
import concourse.tile as tile
from concourse import bacc
from concourse import bass_utils

B, N, C = 4, 2048, 1024
H, D = 16, 64
T = B * N                 # 8192 tokens
NCORES = 8
HPC = H // NCORES         # heads per core = 2
SCALE = D ** -0.5

F32 = mybir.dt.float32
BF16 = mybir.dt.bfloat16

TS = 512                  # phase-A token tile (free dim)
NTS = T // TS             # 16
CCN = C // 128            # 8 contraction chunks
KC = N // 128             # 16 key chunks per batch
QB = N // 512             # 4 query blocks per batch


def _build_graph(nb=B):
    nc = bacc.Bacc("TRN2", target_bir_lowering=False, debug=False,
                   num_devices=NCORES)
    xT = nc.dram_tensor("xT", [C, T], BF16, kind="ExternalInput").ap()
    # wqk columns: [q_h0 | q_h1 | k_h0 | k_h1], each D wide
    wqk = nc.dram_tensor("wqk", [C, HPC * 2 * D], BF16, kind="ExternalInput").ap()
    wv = nc.dram_tensor("wv", [C, HPC * D], BF16, kind="ExternalInput").ap()
    wp = nc.dram_tensor("wp", [HPC * D, C], BF16, kind="ExternalInput").ap()
    y = nc.dram_tensor("y", [T, C], BF16, kind="ExternalOutput").ap()

    with tile.TileContext(nc) as tc, ExitStack() as ctx:
        const = ctx.enter_context(tc.tile_pool(name="const", bufs=1))
        xpool = ctx.enter_context(tc.tile_pool(name="x", bufs=4))
        probs = ctx.enter_context(tc.tile_pool(name="probs", bufs=5))
        stage = ctx.enter_context(tc.tile_pool(name="stage", bufs=3))
        attnp = ctx.enter_context(tc.tile_pool(name="attn", bufs=6))
        alop = ctx.enter_context(tc.tile_pool(name="attn_lo", bufs=2))
        rpool = ctx.enter_context(tc.tile_pool(name="recip", bufs=2))
        outp = ctx.enter_context(tc.tile_pool(name="out", bufs=4))
        # PSUM: st (2 banks x 2) + av (1 x 2) + shared mm (1 x 2) = 8 banks
        st_psum = ctx.enter_context(
            tc.tile_pool(name="st", bufs=4, space="PSUM"))
        av_psum = ctx.enter_context(
            tc.tile_pool(name="av", bufs=2, space="PSUM"))
        mm_psum = ctx.enter_context(
            tc.tile_pool(name="mm", bufs=2, space="PSUM"))

        ones = const.tile([65, 64], BF16)
        nc.gpsimd.memset(ones[:], 1.0)

        wqk_sb = const.tile([128, CCN, HPC * 2 * D], BF16)
        wv_sb = const.tile([128, CCN, HPC * D], BF16)
        wp_sb = const.tile([128, C], BF16)
        for cc in range(CCN):
            nc.gpsimd.dma_start(wqk_sb[:, cc, :], wqk[cc * 128:(cc + 1) * 128, :])
            nc.gpsimd.dma_start(wv_sb[:, cc, :], wv[cc * 128:(cc + 1) * 128, :])
        nc.gpsimd.dma_start(wp_sb[:], wp[:, :])

        # per-batch phase-A outputs (separate tiles so phase B of batch b
        # can overlap phase A of batch b+1):
        #   qT/kT: [d, tokens]; head hh lives on partitions hh*64..+63
        #   v: per head, 16 key-tiles of [128 tok, 65] (col 64 = 1.0)
        qT_b, kT_b, v_b = [], [], []
        for b in range(B):
            qT_b.append(const.tile([128, N], BF16, name=f"qTb{b}", tag=f"qT{b}"))
            kT_b.append(const.tile([128, N], BF16, name=f"kTb{b}", tag=f"kT{b}"))
            v_b.append(const.tile([128, HPC, KC, D + 1], BF16, name=f"vb{b}", tag=f"v{b}"))
            nc.gpsimd.memset(v_b[b][:, :, :, D:D + 1], 1.0)

        def phase_A_dma(b, ts):
            # ---- phase A(b) chunk ts: issue x loads (prefetch) ----
            gsl = slice(b * N + ts * TS, b * N + (ts + 1) * TS)
            xt_lo = xpool.tile([128, CCN // 2, TS], BF16,
                               name=f"xlo{b}_{ts}", tag="xlo")
            xt_hi = xpool.tile([128, CCN // 2, TS], BF16,
                               name=f"xhi{b}_{ts}", tag="xhi")
            # two 512KB DMAs per token tile: (p, cc, j) <- xT[cc*128+p, g0+j]
            xTr = xT.rearrange("(cc p) t -> p cc t", p=128)
            nc.sync.dma_start(xt_lo[:], xTr[:, 0:4, gsl])
            if b == 0:
                nc.scalar.dma_start(xt_hi[:], xTr[:, 4:8, gsl])
            else:
                nc.sync.dma_start(xt_hi[:], xTr[:, 4:8, gsl])
            x_tiles[(b, ts)] = (xt_lo, xt_hi)

        def phase_A(b, ts):
            # ---- phase A(b) chunk ts: projections for batch b ----
            if True:
                sl = slice(ts * TS, (ts + 1) * TS)
                xt_lo, xt_hi = x_tiles.pop((b, ts))
                xts = ([xt_lo[:, cc, :] for cc in range(CCN // 2)]
                       + [xt_hi[:, cc, :] for cc in range(CCN // 2)])
                for qk_i, dst in ((0, qT_b[b]), (1, kT_b[b])):
                    ps = mm_psum.tile([128, TS], F32, tag="mm")
                    for cc in range(CCN):
                        nc.tensor.matmul(
                            ps[:],
                            wqk_sb[:, cc, qk_i * 128:(qk_i + 1) * 128],
                            xts[cc],
                            start=(cc == 0), stop=(cc == CCN - 1))
                    nc.vector.tensor_copy(dst[:, sl], ps[:])
                for j in range(TS // 128):
                    vp = mm_psum.tile([128, HPC * D], F32, tag="mm")
                    for cc in range(CCN):
                        nc.tensor.matmul(
                            vp[:],
                            xts[cc][:, j * 128:(j + 1) * 128],
                            wv_sb[:, cc, :],
                            start=(cc == 0), stop=(cc == CCN - 1))
                    for hh in range(HPC):
                        nc.vector.tensor_copy(
                            v_b[b][:, hh, ts * 4 + j, 0:D],
                            vp[:, hh * D:(hh + 1) * D])

        def phase_B(b, qb):
            # ---- phase B(b) query block qb (attention only; proj deferred)
            if True:
                q0 = qb * 512
                attn = attnp.tile([128, 512], BF16, tag="attn")
                attn_tiles[(b, qb)] = attn
                for hh in (1, 0):
                    h0 = hh * 64
                    av_t = av_psum.tile([D + 1, 512], F32, tag="av")
                    for kc in range(KC):
                        st = st_psum.tile([128, 512], F32, tag="st")
                        pb = probs.tile([128, 512], BF16, tag="probs")
                        k0 = kc * 128
                        nc.tensor.matmul(
                            st[:],
                            kT_b[b][h0:h0 + 64, k0:k0 + 128],
                            qT_b[b][h0:h0 + 64, q0:q0 + 512],
                            start=True, stop=True)
                        nc.scalar.activation(
                            pb[:], st[:], mybir.ActivationFunctionType.Exp,
                            scale=SCALE)
                        nc.tensor.matmul(
                            av_t[:],
                            v_b[b][:, hh, kc, :],
                            pb[:],
                            start=(kc == 0),
                            stop=(kc == KC - 1))
                    rcb = rpool.tile([D + 1, 512], BF16, tag="recipb")
                    with nc.allow_low_precision("softmax denom in bf16"):
                        nc.vector.reciprocal(rcb[D:D + 1, :], av_t[D:D + 1, :])
                    bc = av_psum.tile([64, 512], F32, tag="av")
                    nc.tensor.matmul(bc[:], ones[D:D + 1, :],
                                     rcb[D:D + 1, :],
                                     start=True, stop=True)
                    sg = stage.tile([64, 512], F32, tag="stage")
                    nc.vector.tensor_copy(sg[:], av_t[0:D, :])
                    if hh == 0:
                        nc.vector.tensor_mul(attn[0:64, :], sg[:], bc[:])
                    else:
                        alo = alop.tile([64, 512], BF16, tag="attn_lo")
                        nc.vector.tensor_mul(alo[:], sg[:], bc[:])
                        # engines are partition-aligned; DMA moves the head-1
                        # rows up to partitions 64-127
                        nc.sync.dma_start(attn[h0:h0 + 64, :], alo[:])
        def phase_P(b, only_qb=None):
            # ---- deferred output projection for batch b, interleaved into
            # the next batch so its PE work fills ACT-bound bubbles ----
            for qb in (range(QB) if only_qb is None else [only_qb]):
                attn = attn_tiles.pop((b, qb))
                q0 = qb * 512
                for tt in range(4):
                    ot = outp.tile([128, C], BF16, tag="out")
                    for ob in range(2):
                        pp = mm_psum.tile([128, 512], F32, tag="mm")
                        nc.tensor.matmul(
                            pp[:],
                            attn[:, tt * 128:(tt + 1) * 128],
                            wp_sb[:, ob * 512:(ob + 1) * 512],
                            start=True, stop=True)
                        nc.vector.tensor_copy(ot[:, ob * 512:(ob + 1) * 512],
                                              pp[:])
                    nc.sync.dma_start(
                        y[b * N + q0 + tt * 128:b * N + q0 + (tt + 1) * 128, :],
                        ot[:])

        # software-pipelined emission: keep phase A one batch ahead,
        # interleaved per query-block so its PE work fills the ACT-bound
        # bubbles of phase B uniformly
        attn_tiles = {}
        x_tiles = {}
        for ts in range(N // TS):
            phase_A_dma(0, ts)
        for ts in range(N // TS):
            phase_A(0, ts)
        for b in range(nb):
            for qb in range(QB):
                if b + 1 < nb:
                    # issue the x loads one query-block ahead of their matmuls
                    phase_A_dma(b + 1, qb)
                    if qb > 0:
                        phase_A(b + 1, qb - 1)
                if b > 0:
                    phase_P(b - 1, only_qb=qb)
                phase_B(b, qb)
            if b + 1 < nb:
                phase_A(b + 1, QB - 1)
        phase_P(nb - 1)

    nc.compile()
    return nc


_NC = None
LAST_EXEC_NS = None


def _get_nc():
    global _NC
    if _NC is None:
        _NC = _build_graph()
    return _NC


def _make_in_maps(x, W_qkv, W_proj):
    bf = ml_dtypes.bfloat16
    xT = np.ascontiguousarray(x.reshape(T, C).T.astype(bf))
    in_maps = []
    for i in range(NCORES):
        h0 = HPC * i
        # columns: q_h0 | q_h1 | k_h0 | k_h1
        wqk_i = np.concatenate(
            [W_qkv[(h0 + hh) * D:(h0 + hh + 1) * D, :].T for hh in range(HPC)]
            + [W_qkv[C + (h0 + hh) * D:C + (h0 + hh + 1) * D, :].T
               for hh in range(HPC)],
            axis=1)                                   # [C, HPC*2*D]
        wv_i = W_qkv[2 * C + h0 * D:2 * C + (h0 + HPC) * D, :].T  # [C, HPC*D]
        wp_i = W_proj[:, h0 * D:(h0 + HPC) * D].T     # [HPC*D, C]
        in_maps.append({
            "xT": xT,
            "wqk": np.ascontiguousarray(wqk_i.astype(bf)),
            "wv": np.ascontiguousarray(wv_i.astype(bf)),
            "wp": np.ascontiguousarray(wp_i.astype(bf)),
        })
    return in_maps


def kernel(x, W_qkv, W_proj, b_proj, trace=False):
    global LAST_EXEC_NS
    x = np.ascontiguousarray(np.asarray(x, dtype=np.float32))
    W_qkv = np.asarray(W_qkv, dtype=np.float32)
    W_proj = np.asarray(W_proj, dtype=np.float32)
    b_proj = np.asarray(b_proj, dtype=np.float32)

    in_maps = _make_in_maps(x, W_qkv, W_proj)
    nc = _get_nc()
    res = None
    for attempt in range(3):
        try:
            res = bass_utils.run_bass_kernel_spmd(
                nc, in_maps, core_ids=list(range(NCORES)), trace=trace)
            break
        except Exception:
            # transient "mesh desynced / NRT_EXEC_UNIT_UNRECOVERABLE" errors
            # clear on retry
            if attempt == 2:
                raise
            import time
            time.sleep(5)
    LAST_EXEC_NS = res.exec_time_ns
    acc = res.results[0]["y"].astype(np.float64)
    for i in range(1, NCORES):
        acc += res.results[i]["y"]
    out = (acc + b_proj).astype(np.float32)
    return out.reshape(B, N, C)


def bench(x, W_qkv, W_proj, b_proj, iters=10):
    """Device-resident repeat timing of the NEFF execution.

    Returns (per_iter_ns_blocking, per_iter_ns_pipelined, full output).
    """
    x = np.ascontiguousarray(np.asarray(x, dtype=np.float32))
    in_maps = _make_in_maps(x, np.asarray(W_qkv, dtype=np.float32),
                            np.asarray(W_proj, dtype=np.float32))
    t_block, t_pipe, y_percore = _bench_impl(in_maps, iters=iters)
    acc = y_percore[0].astype(np.float64)
    for i in range(1, NCORES):
        acc += y_percore[i]
    out = (acc + np.asarray(b_proj, dtype=np.float32)).astype(np.float32)
    return t_block, t_pipe, out.reshape(B, N, C)


def _bench_impl(in_maps, iters=10, nc=None):
    import time
    import jax
    from jax.experimental.shard_map import shard_map
    from jax.sharding import Mesh, PartitionSpec, NamedSharding
    from concourse import bass2jax, mybir as mb

    nc = nc or _get_nc()
    bass2jax.install_neuronx_cc_hook()

    partition_name = (nc.partition_id_tensor.name
                      if nc.partition_id_tensor else None)
    in_names, out_names, out_avals, zero_outs = [], [], [], []
    for alloc in nc.m.functions[0].allocations:
        if not isinstance(alloc, mb.MemoryLocationSet):
            continue
        name = alloc.memorylocations[0].name
        if alloc.kind == "ExternalInput":
            if name != partition_name:
                in_names.append(name)
        elif alloc.kind == "ExternalOutput":
            out_names.append(name)
            shape = tuple(alloc.tensor_shape)
            dtype = mb.dt.np(alloc.dtype)
            out_avals.append(jax.core.ShapedArray(shape, dtype))
            zero_outs.append(np.zeros(shape, dtype))
    n_params = len(in_names)
    all_names = in_names + out_names
    if partition_name is not None:
        all_names = all_names + [partition_name]

    def _body(*args):
        operands = list(args)
        if partition_name is not None:
            operands.append(bass2jax.partition_id_tensor())
        outs = bass2jax._bass_exec_p.bind(
            *operands,
            out_avals=tuple(out_avals),
            in_names=tuple(all_names),
            out_names=tuple(out_names),
            lowering_input_output_aliases=(),
            sim_require_finite=True,
            sim_require_nnan=True,
            nc=nc,
        )
        return tuple(outs)

    devices = jax.devices()[:NCORES]
    mesh = Mesh(np.asarray(devices), ("core",))
    spec = PartitionSpec("core")
    sharded = jax.jit(
        shard_map(_body, mesh=mesh,
                  in_specs=(spec,) * (n_params + len(out_names)),
                  out_specs=(spec,) * len(out_names),
                  check_rep=False),
        keep_unused=True)

    shd = NamedSharding(mesh, spec)
    concat_in = [
        np.concatenate([np.asarray(in_maps[c][nm]) for c in range(NCORES)],
                       axis=0) for nm in in_names]
    concat_zero = [np.zeros((NCORES * z.shape[0], *z.shape[1:]), z.dtype)
                   for z in zero_outs]
    dev_in = [jax.device_put(a, shd) for a in concat_in]
    dev_zero = [jax.device_put(a, shd) for a in concat_zero]

    out = sharded(*dev_in, *dev_zero)           # warm-up / compile
    jax.block_until_ready(out)
    if iters == 0:
        return (sharded, dev_in, dev_zero, out_names)

    t_block = []
    for _ in range(iters):
        t0 = time.perf_counter()
        out = sharded(*dev_in, *dev_zero)
        jax.block_until_ready(out)
        t_block.append(time.perf_counter() - t0)

    t0 = time.perf_counter()
    outs = [sharded(*dev_in, *dev_zero) for _ in range(iters)]
    jax.block_until_ready(outs)
    t_pipe = (time.perf_counter() - t0) / iters

    y_global = np.asarray(out[out_names.index("y")])
    return (min(t_block) * 1e9, t_pipe * 1e9,
            y_global.reshape(NCORES, -1, y_global.shape[-1]))

